# revision 20
# baseline (speedup 1.0000x reference)
"""Haar 2D DWT (pywt 'haar') Trainium2 Bass kernel.

Full input x: [16, 64, 256, 256] f32.
Output: [16, 256, 128, 128] f32 = concat(ll, lh, hl, hh) on channel axis.

Sharding: pure data-parallel over batch (16 -> 2 per core x 8 cores).

Per-core layout: partition p = image index (b*C + c) -- exactly 128
images per core. Free dim = a band of R row-pairs of that image
(R*512 contiguous f32 per partition per load). This makes every DMA
descriptor a large contiguous run (R*2KB in, R*512B out per partition).

Per band:
  - stage 0: scale whole tile by 0.5 (DVE tensor_scalar, 2x_2P f32 mode)
  - stage 1 (row butterfly): s = even_row + odd_row, d = even_row - odd_row
  - stage 2 (col butterfly): ll = s_e + s_o, lh = d_e + d_o,
                             hl = s_e - s_o, hh = d_e - d_o
"""

import numpy as np

N_CORES = 8
FULL_B, C, H, W = 16, 64, 256, 256


def _build_bass(B=2, Cc=64, Hh=256, Ww=256, R=16, bufs=2, in_bufs=2, sd_bufs=1,
                out_ring="scalar", split_in=1, taper=0):
    import concourse.bacc as bacc
    import concourse.mybir as mybir
    from concourse.tile import TileContext

    P = B * Cc           # partitions = images per core
    HP = Hh // 2         # row pairs per image
    Wh = Ww // 2
    f32 = mybir.dt.float32
    assert HP % R == 0

    nc = bacc.Bacc("TRN2", target_bir_lowering=False, debug=False)
    x = nc.dram_tensor("x", [B, Cc, Hh, Ww], f32, kind="ExternalInput").ap()
    y = nc.dram_tensor("y", [B, 4 * Cc, HP, Wh], f32, kind="ExternalOutput").ap()

    # [C, B, H*W]: one whole image per partition row, c-major partition
    # order so every DMA's outermost AP dim has count C (HWDGE fans a DMA
    # out across SDMA engines by the outer dim -- count B=2 would use 2
    # of 16 engines).
    xi = x.rearrange("b c h w -> c b (h w)")
    # [4, C, B, HP*Wh]: quadrant-major view of the output images
    yo = y.rearrange("b (q c) h w -> q c b (h w)", q=4)

    with TileContext(nc) as tc:
        with tc.tile_pool(name="pool", bufs=bufs) as pool:
            assert R % split_in == 0
            # Optionally split the last `taper` full bands in half: the
            # final band's stage2->scale->out chain is serial tail latency,
            # so smaller last bands shorten it.
            bands = [R] * (HP // R - taper) + [R // 2] * (2 * taper)
            r0s = [sum(bands[:i]) for i in range(len(bands))]
            for r0, R in zip(r0s, bands):
                Rs = R // split_in
                # s/d are written and read only by DVE, whose program order
                # already serializes reuse -- bufs=1 is free.
                s_t = pool.tile([P, R * Ww], f32, tag="s", bufs=sd_bufs)
                d_t = pool.tile([P, R * Ww], f32, tag="d", bufs=sd_bufs)
                o_ts = [
                    pool.tile([P, R * Wh], f32, name=f"o{q}", tag=f"o{q}")
                    for q in range(4)
                ]
                for h in range(split_in):
                    rh = r0 + h * Rs
                    in_t = pool.tile([P, Rs * Ww * 2], f32, tag="in", bufs=in_bufs)
                    nc.sync.dma_start(
                        out=in_t[:], in_=xi[:, :, rh * 2 * Ww : (rh + Rs) * 2 * Ww]
                    )
                    iv = in_t[:].rearrange("p (r t w) -> p r t w", r=Rs, t=2)
                    sl = slice(h * Rs * Ww, (h + 1) * Rs * Ww)
                    sv = s_t[:, sl].rearrange("p (r w) -> p r w", r=Rs)
                    dv = d_t[:, sl].rearrange("p (r w) -> p r w", r=Rs)
                    nc.vector.tensor_add(out=sv, in0=iv[:, :, 0, :], in1=iv[:, :, 1, :])
                    nc.vector.tensor_sub(out=dv, in0=iv[:, :, 0, :], in1=iv[:, :, 1, :])
                    sp = s_t[:, sl].rearrange("p (r w t) -> p r w t", r=Rs, t=2)
                    dp = d_t[:, sl].rearrange("p (r w t) -> p r w t", r=Rs, t=2)
                    slo = slice(h * Rs * Wh, (h + 1) * Rs * Wh)
                    ovs = [
                        o[:, slo].rearrange("p (r w) -> p r w", r=Rs) for o in o_ts
                    ]
                    nc.vector.tensor_add(
                        out=ovs[0], in0=sp[:, :, :, 0], in1=sp[:, :, :, 1]
                    )
                    nc.vector.tensor_add(
                        out=ovs[1], in0=dp[:, :, :, 0], in1=dp[:, :, :, 1]
                    )
                    nc.vector.tensor_sub(
                        out=ovs[2], in0=sp[:, :, :, 0], in1=sp[:, :, :, 1]
                    )
                    nc.vector.tensor_sub(
                        out=ovs[3], in0=dp[:, :, :, 0], in1=dp[:, :, :, 1]
                    )
                out_eng = {"scalar": nc.scalar, "gpsimd": nc.gpsimd}[out_ring]
                for q in range(4):
                    # Scale-by-0.5 on ACT right before the out-DMA. The out
                    # DMAs ride a ring other than SP so their stage-2 waits
                    # never block the next band's input DMA.
                    nc.scalar.mul(o_ts[q][:], o_ts[q][:], 0.5)
                    out_eng.dma_start(
                        out=yo[q][:, :, r0 * Wh : (r0 + R) * Wh], in_=o_ts[q][:]
                    )
    nc.compile()
    return nc


def kernel(x: np.ndarray) -> np.ndarray:
    from concourse.bass_utils import run_bass_kernel_spmd

    x = np.ascontiguousarray(np.asarray(x, dtype=np.float32))
    assert x.shape == (FULL_B, C, H, W), x.shape
    nc = _build_bass()
    shards = np.split(x, N_CORES, axis=0)
    in_maps = [{"x": s} for s in shards]
    res = run_bass_kernel_spmd(nc, in_maps, list(range(N_CORES)))
    return np.concatenate([r["y"] for r in res.results], axis=0)
